# revision 36
# baseline (speedup 1.0000x reference)
"""Multi-head cross-attention kernel for Trainium2, 8 NeuronCores.

Problem: nn_MultiHeadAttention (H=32 heads, B=8, Lq=Lk=1024, E=128, D=512).

    keys   = einsum('bkd,hde->hbke', states, Wk) + bk
    values = einsum('bkd,hde->hbke', states, Wv) + bv
    attn   = softmax(einsum('bqe,hbke->hbqk', query, keys) / sqrt(E))
    ctx    = einsum('hbqk,hbke->hbqe', attn, values)  -> concat heads
    out    = ctx @ Wo + bo

Sharding: data parallel over batch B=8 -> one batch element per core; no
collectives needed.

Math restructuring (exact algebra, host-side in fp64):
 - bk dropped: softmax(S + const-per-row) == softmax(S);
 - bv folded into the output bias: bo' = bo + sum_h bv[h] @ Wo[h];
 - Wo folded into the value projection: U[h] = Wv[h] @ Wo[h], so
   out = sum_h softmax_h @ (states @ U[h]) + bo'.  This removes the output
   projection from the device entirely.

Per-core dataflow, bf16 matmul operands (same PE rate as fp32r, but the
psum->sbuf copies and rowsum adds run cheaper), N=512 moving operands:

  kt[h]  = Wk[h]-chunks @ states^T    [E, Lk]    8 matmuls
  v'[4h] = states^T-blk @ U-packed    [Lk-c, 4E] 8 matmuls / head
  S^T    = kt-block @ query^T         [Lk-c, Lq] 16 matmuls
  P      = exp(S^T/sqrt(E)) -> bf16   (ACT; scores are O(4) so exp without
                                       max-subtraction is safe in fp32)
  racc   = running sum of 8 P chunks  (7 DVE bf16 adds, ~700ns each)
  rowsum = ones[128,128] @ racc       2 matmuls -- replaces the 16/head the
                                       naive form spends (24% of PE time)
  ctx^T  = v'-chunk @ P-chunks        [E, Lq] psum accum, 16 matmuls
  out_h  = copy(ctx^T) * recip(rowsum)  (DVE, one head behind)
  out    = binary-counter bf16 tree over out_h + bo'  (~1 DVE add/head)

Schedule notes (the measured-on-HW load-bearing decisions):
 - 50 N=512 matmuls/head ~= 10.8us is the PE floor; ACT (8 exps + 2 V'
   copies ~= 10.0us) and DVE (~10us) run just under it.
 - ps_sh (4 banks) holds ONLY the S psum tiles: with the next head's
   first S prefetched at lk7, S allocations are exactly 8 per head, so
   the two buffers alternate cleanly and S(lk+1) never WAR-waits a
   foreign tile's copy.  K/V'/rowsum psum tiles live in a separate
   2x1-bank pool; the AV accumulator keeps its own 2 banks.
 - K projection is emitted in 2-matmul quarters at lk0-3, V' chunks at
   lk4/lk6, the previous head's rowsum at lk5 and its normalization at
   lk6, so every iteration carries >=1.4us of PE work and the exp chain
   (~1.1us + sync) never gates an AV matmul.
 - ps_c is freed by a copy emitted BEFORE the last racc add (DVE FIFO
   order), so the next head's first AV matmul never waits.
 - GpSimd is kept completely idle: it shares an SBUF port with DVE and a
   single in-flight GpSimd op slows concurrent DVE ops ~3.5x.
 - The last head pulls rowsum/norm/tree-merge forward and the epilogue
   normalizes straight from psum in halves, overlapping the output DMA.
"""

import numpy as np
import ml_dtypes

import concourse.bass as bass
import concourse.mybir as mybir
import concourse.tile as tile
from concourse import bacc
from concourse.bass_utils import run_bass_kernel_spmd

H, E, D = 32, 128, 512
B, LQ, LK = 8, 1024, 1024
NDC = D // 128    # 4 contraction chunks for the projections
NLK = LK // 128   # 8 key chunks
HPG = 4           # heads per group for the packed V' computation
NG = H // HPG
SCALE = 1.0 / float(np.sqrt(E))

F32 = mybir.dt.float32
F32R = mybir.dt.float32r
BF16 = mybir.dt.bfloat16
EXP = mybir.ActivationFunctionType.Exp
COPY = mybir.ActivationFunctionType.Copy

N_CORES = 8


def _round_f32r(a):
    """Round fp32 -> fp32r (11-bit mantissa, low 12 bits zero), RN-even."""
    b = np.ascontiguousarray(a, dtype=np.float32).view(np.uint32)
    b = b + 0x7FF + ((b >> 12) & 1)
    b &= np.uint32(0xFFFFF000)
    return b.view(np.float32)


def _build_kernel(tc, qT, sT, wk, u, ones, bo2, outT):
    nc = tc.nc
    with (
        tc.tile_pool(name="const", bufs=1) as cpool,
        tc.tile_pool(name="wkp", bufs=2) as wkp,
        tc.tile_pool(name="up", bufs=2) as upool,
        tc.tile_pool(name="ktp", bufs=2) as ktp,
        tc.tile_pool(name="vp", bufs=2) as vpool,
        tc.tile_pool(name="pp", bufs=4) as ppool,
        tc.tile_pool(name="rap", bufs=3) as rapool,
        tc.tile_pool(name="normp", bufs=2) as npool,
        tc.tile_pool(name="ps_sh", bufs=2, space="PSUM") as ps_sh,
        tc.tile_pool(name="ps_acc", bufs=1, space="PSUM") as ps_acc,
        tc.tile_pool(name="ps_x2", bufs=2, space="PSUM") as ps_x2,
    ):
        # ---- resident inputs; st chunks are separate tiles so the first
        # projection matmul only waits on the first quarter of the DMA ----
        st_sb = [cpool.tile([128, LK], BF16, name=f"st{c}")
                 for c in range(NDC)]
        nc.sync.dma_start(st_sb[0][:], sT[0:128, :])
        q_sb = cpool.tile([E, LQ], BF16)
        ones_sb = cpool.tile([128, 128], BF16)
        bo2_sb = cpool.tile([E, 1], F32)
        out_acc = cpool.tile([E, LQ], F32)

        kt_by_head = {}
        wk_by_head = {}

        def emit_k_dma(h):
            wk_sb = wkp.tile([128, NDC, E], BF16, tag="wk", name="wk_sb")
            for c in range(NDC):
                nc.sync.dma_start(wk_sb[:, c, :],
                                  wk[h, c * 128:(c + 1) * 128, :])
            wk_by_head[h] = wk_sb

        kproj_state = {}

        def emit_k_part(h, part):
            """Quarter of head h's K^T projection: two of the eight
            contraction matmuls, so four consecutive iterations each get
            one ~1us filler. Halves are copied out on DVE as they finish."""
            half, sub = divmod(part, 2)
            if part == 0:
                wk_sb = wk_by_head.pop(h)
                kt_sb = ktp.tile([E, LK], BF16, tag="kt", name="kt_sb")
                kproj_state[h] = [wk_sb, kt_sb, None]
            wk_sb, kt_sb, ps_k = kproj_state[h]
            sl = bass.ts(half, 512)
            if sub == 0:
                ps_k = ps_x2.tile([E, 512], F32, tag="x2", name="ps_k")
                kproj_state[h][2] = ps_k
            for c in range(2 * sub, 2 * sub + 2):
                nc.tensor.matmul(ps_k[:], wk_sb[:, c, :], st_sb[c][:, sl],
                                 start=(c == 0), stop=(c == NDC - 1))
            if sub == 1:
                nc.vector.tensor_copy(kt_sb[:, sl], ps_k[:])
            if part == 3:
                del kproj_state[h]
                kt_by_head[h] = kt_sb

        def emit_k(h):
            for part in range(4):
                emit_k_part(h, part)

        # ---- rowsum + normalization, pipelined one head behind ----
        pending_tail = {}   # h -> (racc, ctx_raw)
        pending_norm = {}   # h -> (ctx_raw, recip)

        def emit_rowsum_prev():
            if not pending_tail:
                return
            (h, (racc, ctx_raw)), = pending_tail.items()
            pending_tail.clear()
            recip_sb = npool.tile([128, LQ], F32, tag="recip",
                                  name="recip_sb")
            for half in range(2):
                sl = bass.ts(half, 512)
                ps_r = ps_x2.tile([128, 512], F32, tag="x2", name="ps_r")
                nc.tensor.matmul(ps_r[:], ones_sb[:], racc[:, sl],
                                 start=True, stop=True)
                nc.vector.reciprocal_approx_fast(recip_sb[:, sl], ps_r[:])
            pending_norm[h] = (ctx_raw, recip_sb)

        # Per-head outputs are accumulated in a bf16 binary-counter tree on
        # DVE (~1 add per head amortized).  GpSimd is kept completely idle:
        # its SBUF port is shared with DVE and a single in-flight GpSimd op
        # was measured to slow concurrent DVE ops ~3.5x.
        acc_levels = [None] * 6

        def counter_accumulate(x):
            for lv in range(6):
                if acc_levels[lv] is None:
                    acc_levels[lv] = x
                    return
                y = npool.tile([E, LQ], BF16, tag=f"acc{lv}",
                               name=f"acc{lv}")
                nc.vector.tensor_add(y[:], acc_levels[lv][:], x[:])
                acc_levels[lv] = None
                x = y
            raise AssertionError("counter overflow")

        def merge_levels():
            live = [t for t in acc_levels if t is not None]
            x = live[0]
            for i, t in enumerate(live[1:]):
                y = npool.tile([E, LQ], BF16, tag="merge", name="merge")
                nc.vector.tensor_add(y[:], x[:], t[:])
                x = y
            acc_levels[:] = [None] * 6
            return x

        def emit_norm():
            if not pending_norm:
                return
            (h, (ctx_raw, recip_sb)), = pending_norm.items()
            pending_norm.clear()
            ctxn = npool.tile([E, LQ], BF16, tag="ctxn", name="ctxn")
            nc.vector.tensor_mul(ctxn[:], ctx_raw[:], recip_sb[:])
            counter_accumulate(ctxn)

        # ---- prologue: interleave st/wk chunk DMAs so Kproj(0)'s
        # accumulation can start as soon as the first pair lands ----
        wk_sb0 = wkp.tile([128, NDC, E], BF16, tag="wk", name="wk_sb")
        nc.sync.dma_start(wk_sb0[:, 0, :], wk[0, 0:128, :])
        for c in range(1, NDC):
            nc.sync.dma_start(st_sb[c][:], sT[c * 128:(c + 1) * 128, :])
            nc.sync.dma_start(wk_sb0[:, c, :], wk[0, c * 128:(c + 1) * 128, :])
        wk_by_head[0] = wk_sb0
        u_sb0 = upool.tile([128, NDC, HPG * E], BF16, tag="u", name="u_sb")
        for c in range(NDC):
            nc.sync.dma_start(u_sb0[:, c, :], u[c * 128:(c + 1) * 128,
                                                0:HPG * E])
        nc.sync.dma_start(q_sb[:], qT[:])
        nc.sync.dma_start(ones_sb[:], ones[:])
        nc.sync.dma_start(bo2_sb[:], bo2[:])
        emit_k(0)
        emit_k_dma(1)

        vstate = {}

        def emit_vchunk(g, lk, part=None):
            """One lk-chunk of group g's packed V' projection (4 matmuls +
            ACT copy). Chunks are spread through the head loop to keep the
            PE ahead of the exp chain; part 0/1 emits just two of the four
            contraction matmuls so a chunk can fill two thin iterations."""
            if part in (None, 0):
                vstate["ps"] = ps_x2.tile([128, HPG * E], F32, tag="x2",
                                          name="ps_v")
            ps_v = vstate["ps"]
            cs = {None: range(NDC), 0: range(2), 1: range(2, NDC)}[part]
            for c in cs:
                nc.tensor.matmul(
                    ps_v[:], st_sb[c][:, lk * 128:(lk + 1) * 128],
                    vstate[("u", g)][:, c, :],
                    start=(c == 0), stop=(c == NDC - 1))
            if part in (None, 1):
                nc.scalar.activation(vstate[("v", g)][:, lk, :], ps_v[:],
                                     COPY)

        def prep_group(g, u_sb=None):
            if u_sb is None:
                u_sb = upool.tile([128, NDC, HPG * E], BF16, tag="u",
                                  name="u_sb")
                for c in range(NDC):
                    nc.sync.dma_start(
                        u_sb[:, c, :],
                        u[c * 128:(c + 1) * 128,
                          g * HPG * E:(g + 1) * HPG * E])
            vstate[("u", g)] = u_sb
            vstate[("v", g)] = vpool.tile([128, NLK, HPG * E], BF16,
                                          tag="v", name="v_sb")

        prep_group(0, u_sb0)
        for lk in range(4):
            emit_vchunk(0, lk)

        def emit_s(kt_sb, lk):
            ps_s = ps_sh.tile([128, LQ], F32, tag="sh", name="ps_s")
            for half in range(2):
                sl = bass.ts(half, 512)
                nc.tensor.matmul(ps_s[:, sl],
                                 kt_sb[:, lk * 128:(lk + 1) * 128],
                                 q_sb[:, sl], start=True, stop=True)
            p_sb = ppool.tile([128, LQ], BF16, tag="p", name="p_sb")
            nc.scalar.activation(p_sb[:], ps_s[:], EXP, scale=SCALE)
            return p_sb

        s0_next = [None]

        for g in range(NG):
            v_sb = vstate[("v", g)]
            if g + 1 < NG and g == 0:
                prep_group(g + 1)

            for hh in range(HPG):
                h = g * HPG + hh
                kt_sb = kt_by_head.pop(h)

                ps_c = ps_acc.tile([E, LQ], F32, tag="c", name="ps_c")
                if s0_next[0] is None:
                    p_next = emit_s(kt_sb, 0)
                else:
                    p_next = s0_next[0]
                    s0_next[0] = None
                racc = None
                for lk in range(NLK):
                    p_cur = p_next
                    if lk + 1 < NLK:
                        p_next = emit_s(kt_sb, lk + 1)
                    elif h + 1 < H:
                        # prefetch the next head's first S matmul into this
                        # otherwise-thin iteration: its exp gets a full
                        # iteration of slack before the boundary
                        s0_next[0] = emit_s(kt_by_head[h + 1], 0)
                    # fillers: one ~1us projection lump per iteration keeps
                    # the PE ahead of the exp chain everywhere
                    if h == 0:
                        # bootstrap: JIT group 0's remaining chunks, then
                        # head 1's K projection, then group 1's first chunk
                        if lk < 4:
                            emit_vchunk(0, lk + 4)
                        elif lk in (4, 5):
                            emit_k_part(1, 2 * (lk - 4))
                            emit_k_part(1, 2 * (lk - 4) + 1)
                            if lk == 4:
                                emit_k_dma(2)
                        elif lk == 6:
                            emit_vchunk(1, 0)
                        elif lk == 7:
                            emit_vchunk(1, 1)
                    else:
                        if lk < 4:
                            if h + 1 < H:
                                emit_k_part(h + 1, lk)
                                if lk == 0 and h + 2 < H:
                                    emit_k_dma(h + 2)
                        elif lk in (4, 6):
                            if g + 1 < NG and not (hh == 0 and g == 0):
                                emit_vchunk(g + 1, 2 * hh + (lk - 4) // 2)
                        if lk == 1 and hh == 0 and g + 1 < NG and g > 0:
                            prep_group(g + 1)
                        # final head: pull the previous head's rowsum/norm
                        # and the output-tree merge forward so the epilogue
                        # only needs one short DVE chain
                        if lk == (1 if h == H - 1 else 5):
                            emit_rowsum_prev()
                        if lk == (2 if h == H - 1 else 6):
                            emit_norm()
                            if h == H - 2:
                                # pre-merge heads 0..30's output tree here
                                # (DVE slack); head 31 then needs one add
                                acc_levels[5] = merge_levels()
                            if h == H - 1:
                                merged = merge_levels()
                    for half in range(2):
                        sl = bass.ts(half, 512)
                        nc.tensor.matmul(ps_c[:, sl],
                                         v_sb[:, lk, hh * E:(hh + 1) * E],
                                         p_cur[:, sl],
                                         start=(lk == 0), stop=(lk == NLK - 1))
                    # free ps_c the moment the last AV matmul retires:
                    # the copy is emitted BEFORE the final racc add so it
                    # does not queue behind it on DVE (the next head's
                    # first AV matmul WAR-waits on this copy)
                    if lk == NLK - 1 and h != H - 1:
                        ctx_raw = npool.tile([E, LQ], BF16, tag="ctx",
                                             name="ctx_raw")
                        nc.vector.tensor_copy(ctx_raw[:], ps_c[:])
                    # rowsum running accumulation on DVE (bf16 2x mode,
                    # ~700ns each, paced one per iteration by the exps)
                    if lk == 0:
                        racc = p_cur
                    else:
                        racc_new = rapool.tile([128, LQ], BF16, tag="racc",
                                               name="racc")
                        nc.vector.tensor_add(racc_new[:], racc[:], p_cur[:])
                        racc = racc_new

                # rowsum/norm happen next head; the final head's
                # normalization reads the psum tile directly in the epilogue
                if h == H - 1:
                    pending_tail[h] = (racc, ps_c)
                else:
                    pending_tail[h] = (racc, ctx_raw)

        # ---- epilogue: head 31's rowsum, then a short half-pipelined DVE
        # chain (normalize from psum, add the merged tree, bias, DMA out)
        emit_rowsum_prev()
        (hl, (ps_c31, recip31)), = pending_norm.items()
        pending_norm.clear()
        ctxn31 = npool.tile([E, LQ], BF16, tag="ctxn", name="ctxn")
        tfin = npool.tile([E, LQ], BF16, tag="merge", name="tfin")
        for half in range(2):
            sl = bass.ts(half, 512)
            nc.vector.tensor_mul(ctxn31[:, sl], ps_c31[:, sl],
                                 recip31[:, sl])
            nc.vector.tensor_add(tfin[:, sl], merged[:, sl], ctxn31[:, sl])
            nc.vector.tensor_scalar_add(out_acc[:, sl], tfin[:, sl],
                                        bo2_sb[:, 0:1])
            nc.sync.dma_start(outT[:, sl], out_acc[:, sl])


def build_program():
    nc = bacc.Bacc("TRN2", target_bir_lowering=False, debug=False,
                   num_devices=N_CORES)
    qT = nc.dram_tensor("qT", [E, LQ], BF16, kind="ExternalInput").ap()
    sT = nc.dram_tensor("sT", [D, LK], BF16, kind="ExternalInput").ap()
    wk = nc.dram_tensor("wk", [H, D, E], BF16, kind="ExternalInput").ap()
    u = nc.dram_tensor("u", [D, H * E], BF16, kind="ExternalInput").ap()
    ones = nc.dram_tensor("ones", [128, 128], BF16, kind="ExternalInput").ap()
    bo2 = nc.dram_tensor("bo2", [E, 1], F32, kind="ExternalInput").ap()
    outT = nc.dram_tensor("outT", [E, LQ], F32, kind="ExternalOutput").ap()

    with tile.TileContext(nc) as tc:
        _build_kernel(tc, qT, sT, wk, u, ones, bo2, outT)
    nc.compile()
    return nc


def make_in_maps(query, states, Wk, bk, Wv, bv, Wo, bo):
    """Shard the full inputs into per-core input maps (host-side prep)."""
    bb = ml_dtypes.bfloat16
    WoH = Wo.reshape(H, E, E).astype(np.float64)
    # fold Wo through the value projection and bv through the output bias
    # (softmax rows sum to 1), both exact in fp64
    U = np.einsum('hde,hef->hdf', Wv.astype(np.float64), WoH)
    u_packed = np.ascontiguousarray(
        np.transpose(U, (1, 0, 2)).reshape(D, H * E)).astype(bb)
    bo2 = bo.astype(np.float64) + np.einsum('he,hef->f',
                                            bv.astype(np.float64), WoH)
    bo2 = bo2.astype(np.float32).reshape(E, 1)
    wk_c = np.ascontiguousarray(Wk).astype(bb)
    ones_c = np.ones((128, 128), dtype=bb)

    in_maps = []
    for b in range(B):
        in_maps.append({
            "qT": np.ascontiguousarray(query[b].T).astype(bb),
            "sT": np.ascontiguousarray(states[b].T).astype(bb),
            "wk": wk_c,
            "u": u_packed,
            "ones": ones_c,
            "bo2": bo2,
        })
    return in_maps


_PROGRAM_CACHE = {}


def _get_program():
    if "nc" not in _PROGRAM_CACHE:
        _PROGRAM_CACHE["nc"] = build_program()
    return _PROGRAM_CACHE["nc"]


def kernel(query, states, Wk, bk, Wv, bv, Wo, bo, _trace=False, _tmpdir=None):
    args = [np.asarray(a, dtype=np.float32)
            for a in (query, states, Wk, bk, Wv, bv, Wo, bo)]
    nc = _get_program()
    in_maps = make_in_maps(*args)
    last_err = None
    for _attempt in range(2):  # one retry for transient device errors
        try:
            res = run_bass_kernel_spmd(nc, in_maps,
                                       core_ids=list(range(N_CORES)),
                                       trace=_trace, tmpdir=_tmpdir)
            break
        except Exception as e:  # noqa: BLE001
            last_err = e
    else:
        raise last_err
    out = np.stack([res.results[b]["outT"].T for b in range(B)])
    out = np.ascontiguousarray(out.astype(np.float32))
    if _trace:
        kernel.last_exec_time_ns = res.exec_time_ns
        kernel.last_results = res
    return out


if __name__ == "__main__":
    rng = np.random.default_rng(0)
    inputs = {
        "query": rng.standard_normal((B, LQ, E), dtype=np.float32),
        "states": rng.standard_normal((B, LK, D), dtype=np.float32),
        "Wk": rng.uniform(-0.04, 0.04, (H, D, E)).astype(np.float32),
        "bk": rng.uniform(-0.04, 0.04, (H, E)).astype(np.float32),
        "Wv": rng.uniform(-0.04, 0.04, (H, D, E)).astype(np.float32),
        "bv": rng.uniform(-0.04, 0.04, (H, E)).astype(np.float32),
        "Wo": rng.uniform(-0.015, 0.015, (H * E, E)).astype(np.float32),
        "bo": rng.uniform(-0.015, 0.015, (E,)).astype(np.float32),
    }
    out = kernel(**inputs)
    print(out.shape, out.dtype)


# revision 37
# speedup vs baseline: 1.0063x; 1.0063x over previous
"""Multi-head cross-attention kernel for Trainium2, 8 NeuronCores.

Problem: nn_MultiHeadAttention (H=32 heads, B=8, Lq=Lk=1024, E=128, D=512).

    keys   = einsum('bkd,hde->hbke', states, Wk) + bk
    values = einsum('bkd,hde->hbke', states, Wv) + bv
    attn   = softmax(einsum('bqe,hbke->hbqk', query, keys) / sqrt(E))
    ctx    = einsum('hbqk,hbke->hbqe', attn, values)  -> concat heads
    out    = ctx @ Wo + bo

Sharding: data parallel over batch B=8 -> one batch element per core; no
collectives needed.

Math restructuring (exact algebra, host-side in fp64):
 - bk dropped: softmax(S + const-per-row) == softmax(S);
 - bv folded into the output bias: bo' = bo + sum_h bv[h] @ Wo[h];
 - Wo folded into the value projection: U[h] = Wv[h] @ Wo[h], so
   out = sum_h softmax_h @ (states @ U[h]) + bo'.  This removes the output
   projection from the device entirely.

Per-core dataflow, bf16 matmul operands (same PE rate as fp32r, but the
psum->sbuf copies and rowsum adds run cheaper), N=512 moving operands:

  kt[h]  = Wk[h]-chunks @ states^T    [E, Lk]    8 matmuls
  v'[4h] = states^T-blk @ U-packed    [Lk-c, 4E] 8 matmuls / head
  S^T    = kt-block @ query^T         [Lk-c, Lq] 16 matmuls
  P      = exp(S^T/sqrt(E)) -> bf16   (ACT; scores are O(4) so exp without
                                       max-subtraction is safe in fp32)
  racc   = running sum of 8 P chunks  (7 DVE bf16 adds, ~700ns each)
  rowsum = ones[128,128] @ racc       2 matmuls -- replaces the 16/head the
                                       naive form spends (24% of PE time)
  ctx^T  = v'-chunk @ P-chunks        [E, Lq] psum accum, 16 matmuls
  out_h  = copy(ctx^T) * recip(rowsum)  (DVE, one head behind)
  out    = binary-counter bf16 tree over out_h + bo'  (~1 DVE add/head)

Schedule notes (the measured-on-HW load-bearing decisions):
 - 50 N=512 matmuls/head ~= 10.8us is the PE floor; ACT (8 exps + 2 V'
   copies ~= 10.0us) and DVE (~10us) run just under it.
 - ps_sh (4 banks) holds ONLY the S psum tiles: with the next head's
   first S prefetched at lk7, S allocations are exactly 8 per head, so
   the two buffers alternate cleanly and S(lk+1) never WAR-waits a
   foreign tile's copy.  K/V'/rowsum psum tiles live in a separate
   2x1-bank pool; the AV accumulator keeps its own 2 banks.
 - K projection is emitted in 2-matmul quarters at lk0-3, V' chunks at
   lk4/lk6, the previous head's rowsum at lk5 and its normalization at
   lk6, so every iteration carries >=1.4us of PE work and the exp chain
   (~1.1us + sync) never gates an AV matmul.
 - ps_c is freed by a copy emitted BEFORE the last racc add (DVE FIFO
   order), so the next head's first AV matmul never waits.
 - GpSimd is kept completely idle: it shares an SBUF port with DVE and a
   single in-flight GpSimd op slows concurrent DVE ops ~3.5x.
 - The last head pulls rowsum/norm/tree-merge forward and the epilogue
   normalizes straight from psum in halves, overlapping the output DMA.
"""

import numpy as np
import ml_dtypes

import concourse.bass as bass
import concourse.mybir as mybir
import concourse.tile as tile
from concourse import bacc
from concourse.bass_utils import run_bass_kernel_spmd

H, E, D = 32, 128, 512
B, LQ, LK = 8, 1024, 1024
NDC = D // 128    # 4 contraction chunks for the projections
NLK = LK // 128   # 8 key chunks
HPG = 4           # heads per group for the packed V' computation
NG = H // HPG
SCALE = 1.0 / float(np.sqrt(E))

F32 = mybir.dt.float32
F32R = mybir.dt.float32r
BF16 = mybir.dt.bfloat16
EXP = mybir.ActivationFunctionType.Exp
COPY = mybir.ActivationFunctionType.Copy

N_CORES = 8


def _round_f32r(a):
    """Round fp32 -> fp32r (11-bit mantissa, low 12 bits zero), RN-even."""
    b = np.ascontiguousarray(a, dtype=np.float32).view(np.uint32)
    b = b + 0x7FF + ((b >> 12) & 1)
    b &= np.uint32(0xFFFFF000)
    return b.view(np.float32)


def _build_kernel(tc, qT, sT, wk, u, ones, bo2, outT):
    nc = tc.nc
    with (
        tc.tile_pool(name="const", bufs=1) as cpool,
        tc.tile_pool(name="wkp", bufs=2) as wkp,
        tc.tile_pool(name="up", bufs=2) as upool,
        tc.tile_pool(name="ktp", bufs=2) as ktp,
        tc.tile_pool(name="vp", bufs=2) as vpool,
        tc.tile_pool(name="pp", bufs=4) as ppool,
        tc.tile_pool(name="rap", bufs=3) as rapool,
        tc.tile_pool(name="normp", bufs=2) as npool,
        tc.tile_pool(name="ps_sh", bufs=2, space="PSUM") as ps_sh,
        tc.tile_pool(name="ps_acc", bufs=1, space="PSUM") as ps_acc,
        tc.tile_pool(name="ps_x2", bufs=2, space="PSUM") as ps_x2,
    ):
        # ---- resident inputs; st chunks are separate tiles so the first
        # projection matmul only waits on the first quarter of the DMA ----
        st_sb = [cpool.tile([128, LK], BF16, name=f"st{c}")
                 for c in range(NDC)]
        nc.sync.dma_start(st_sb[0][:], sT[0:128, :])
        q_sb = cpool.tile([E, LQ], BF16)
        ones_sb = cpool.tile([128, 128], BF16)
        bo2_sb = cpool.tile([E, 1], F32)
        out_acc = cpool.tile([E, LQ], F32)

        kt_by_head = {}
        wk_by_head = {}

        def emit_k_dma(h):
            wk_sb = wkp.tile([128, NDC, E], BF16, tag="wk", name="wk_sb")
            for c in range(NDC):
                nc.sync.dma_start(wk_sb[:, c, :],
                                  wk[h, c * 128:(c + 1) * 128, :])
            wk_by_head[h] = wk_sb

        kproj_state = {}

        def emit_k_part(h, part):
            """Quarter of head h's K^T projection: two of the eight
            contraction matmuls, so four consecutive iterations each get
            one ~1us filler. Halves are copied out on DVE as they finish."""
            half, sub = divmod(part, 2)
            if part == 0:
                wk_sb = wk_by_head.pop(h)
                kt_sb = ktp.tile([E, LK], BF16, tag="kt", name="kt_sb")
                kproj_state[h] = [wk_sb, kt_sb, None]
            wk_sb, kt_sb, ps_k = kproj_state[h]
            sl = bass.ts(half, 512)
            if sub == 0:
                ps_k = ps_x2.tile([E, 512], F32, tag="x2", name="ps_k")
                kproj_state[h][2] = ps_k
            for c in range(2 * sub, 2 * sub + 2):
                nc.tensor.matmul(ps_k[:], wk_sb[:, c, :], st_sb[c][:, sl],
                                 start=(c == 0), stop=(c == NDC - 1))
            if sub == 1:
                nc.vector.tensor_copy(kt_sb[:, sl], ps_k[:])
            if part == 3:
                del kproj_state[h]
                kt_by_head[h] = kt_sb

        def emit_k(h):
            for part in range(4):
                emit_k_part(h, part)

        # ---- rowsum + normalization, pipelined one head behind ----
        pending_tail = {}   # h -> (racc, ctx_raw)
        pending_norm = {}   # h -> (ctx_raw, recip)

        def emit_rowsum_prev():
            if not pending_tail:
                return
            (h, (racc, ctx_raw)), = pending_tail.items()
            pending_tail.clear()
            recip_sb = npool.tile([128, LQ], F32, tag="recip",
                                  name="recip_sb")
            for half in range(2):
                sl = bass.ts(half, 512)
                ps_r = ps_x2.tile([128, 512], F32, tag="x2", name="ps_r")
                nc.tensor.matmul(ps_r[:], ones_sb[:], racc[:, sl],
                                 start=True, stop=True)
                nc.vector.reciprocal_approx_fast(recip_sb[:, sl], ps_r[:])
            pending_norm[h] = (ctx_raw, recip_sb)

        # Per-head outputs are accumulated in a bf16 binary-counter tree on
        # DVE (~1 add per head amortized).  GpSimd is kept completely idle:
        # its SBUF port is shared with DVE and a single in-flight GpSimd op
        # was measured to slow concurrent DVE ops ~3.5x.
        acc_levels = [None] * 6

        def counter_accumulate(x):
            for lv in range(6):
                if acc_levels[lv] is None:
                    acc_levels[lv] = x
                    return
                y = npool.tile([E, LQ], BF16, tag=f"acc{lv}",
                               name=f"acc{lv}")
                nc.vector.tensor_add(y[:], acc_levels[lv][:], x[:])
                acc_levels[lv] = None
                x = y
            raise AssertionError("counter overflow")

        def merge_levels():
            live = [t for t in acc_levels if t is not None]
            x = live[0]
            for i, t in enumerate(live[1:]):
                y = npool.tile([E, LQ], BF16, tag="merge", name="merge")
                nc.vector.tensor_add(y[:], x[:], t[:])
                x = y
            acc_levels[:] = [None] * 6
            return x

        def emit_norm():
            if not pending_norm:
                return
            (h, (ctx_raw, recip_sb)), = pending_norm.items()
            pending_norm.clear()
            ctxn = npool.tile([E, LQ], BF16, tag="ctxn", name="ctxn")
            nc.vector.tensor_mul(ctxn[:], ctx_raw[:], recip_sb[:])
            counter_accumulate(ctxn)

        # ---- prologue: interleave st/wk chunk DMAs so Kproj(0)'s
        # accumulation can start as soon as the first pair lands ----
        wk_sb0 = wkp.tile([128, NDC, E], BF16, tag="wk", name="wk_sb")
        nc.sync.dma_start(wk_sb0[:, 0, :], wk[0, 0:128, :])
        for c in range(1, NDC):
            nc.sync.dma_start(st_sb[c][:], sT[c * 128:(c + 1) * 128, :])
            nc.sync.dma_start(wk_sb0[:, c, :], wk[0, c * 128:(c + 1) * 128, :])
        wk_by_head[0] = wk_sb0
        u_sb0 = upool.tile([128, NDC, HPG * E], BF16, tag="u", name="u_sb")
        for c in range(NDC):
            nc.sync.dma_start(u_sb0[:, c, :], u[c * 128:(c + 1) * 128,
                                                0:HPG * E])
        nc.sync.dma_start(q_sb[:], qT[:])
        nc.sync.dma_start(ones_sb[:], ones[:])
        nc.sync.dma_start(bo2_sb[:], bo2[:])
        emit_k(0)
        emit_k_dma(1)

        vstate = {}

        def emit_vchunk(g, lk, part=None):
            """One lk-chunk of group g's packed V' projection (4 matmuls +
            ACT copy). Chunks are spread through the head loop to keep the
            PE ahead of the exp chain; part 0/1 emits just two of the four
            contraction matmuls so a chunk can fill two thin iterations."""
            if part in (None, 0):
                vstate["ps"] = ps_x2.tile([128, HPG * E], F32, tag="x2",
                                          name="ps_v")
            ps_v = vstate["ps"]
            cs = {None: range(NDC), 0: range(2), 1: range(2, NDC)}[part]
            for c in cs:
                nc.tensor.matmul(
                    ps_v[:], st_sb[c][:, lk * 128:(lk + 1) * 128],
                    vstate[("u", g)][:, c, :],
                    start=(c == 0), stop=(c == NDC - 1))
            if part in (None, 1):
                nc.scalar.activation(vstate[("v", g)][:, lk, :], ps_v[:],
                                     COPY)

        def prep_group(g, u_sb=None):
            if u_sb is None:
                u_sb = upool.tile([128, NDC, HPG * E], BF16, tag="u",
                                  name="u_sb")
                for c in range(NDC):
                    nc.sync.dma_start(
                        u_sb[:, c, :],
                        u[c * 128:(c + 1) * 128,
                          g * HPG * E:(g + 1) * HPG * E])
            vstate[("u", g)] = u_sb
            vstate[("v", g)] = vpool.tile([128, NLK, HPG * E], BF16,
                                          tag="v", name="v_sb")

        prep_group(0, u_sb0)
        for lk in range(2):
            emit_vchunk(0, lk)

        def emit_s(kt_sb, lk):
            ps_s = ps_sh.tile([128, LQ], F32, tag="sh", name="ps_s")
            for half in range(2):
                sl = bass.ts(half, 512)
                nc.tensor.matmul(ps_s[:, sl],
                                 kt_sb[:, lk * 128:(lk + 1) * 128],
                                 q_sb[:, sl], start=True, stop=True)
            p_sb = ppool.tile([128, LQ], BF16, tag="p", name="p_sb")
            nc.scalar.activation(p_sb[:], ps_s[:], EXP, scale=SCALE)
            return p_sb

        s0_next = [None]

        for g in range(NG):
            v_sb = vstate[("v", g)]
            if g + 1 < NG and g == 0:
                prep_group(g + 1)

            for hh in range(HPG):
                h = g * HPG + hh
                kt_sb = kt_by_head.pop(h)

                ps_c = ps_acc.tile([E, LQ], F32, tag="c", name="ps_c")
                if s0_next[0] is None:
                    p_next = emit_s(kt_sb, 0)
                else:
                    p_next = s0_next[0]
                    s0_next[0] = None
                racc = None
                for lk in range(NLK):
                    p_cur = p_next
                    if lk + 1 < NLK:
                        p_next = emit_s(kt_sb, lk + 1)
                    elif h + 1 < H:
                        # prefetch the next head's first S matmul into this
                        # otherwise-thin iteration: its exp gets a full
                        # iteration of slack before the boundary
                        s0_next[0] = emit_s(kt_by_head[h + 1], 0)
                    # fillers: one ~1us projection lump per iteration keeps
                    # the PE ahead of the exp chain everywhere
                    if h == 0:
                        # bootstrap: JIT group 0's chunks, then head 1's K
                        if lk == 0 and h + 1 < H:
                            emit_k(1)
                            emit_k_dma(2)
                        if lk < 6:
                            emit_vchunk(0, lk + 2)
                        else:
                            emit_vchunk(1, lk - 6)
                    else:
                        if lk < 4:
                            if h + 1 < H:
                                emit_k_part(h + 1, lk)
                                if lk == 0 and h + 2 < H:
                                    emit_k_dma(h + 2)
                        elif lk in (4, 6):
                            if g + 1 < NG and not (hh == 0 and g == 0):
                                emit_vchunk(g + 1, 2 * hh + (lk - 4) // 2)
                        if lk == 1 and hh == 0 and g + 1 < NG and g > 0:
                            prep_group(g + 1)
                        # final head: pull the previous head's rowsum/norm
                        # and the output-tree merge forward so the epilogue
                        # only needs one short DVE chain
                        if lk == (1 if h == H - 1 else 5):
                            emit_rowsum_prev()
                        if lk == (2 if h == H - 1 else 6):
                            emit_norm()
                            if h == H - 2:
                                # pre-merge heads 0..30's output tree here
                                # (DVE slack); head 31 then needs one add
                                acc_levels[5] = merge_levels()
                            if h == H - 1:
                                merged = merge_levels()
                    for half in range(2):
                        sl = bass.ts(half, 512)
                        nc.tensor.matmul(ps_c[:, sl],
                                         v_sb[:, lk, hh * E:(hh + 1) * E],
                                         p_cur[:, sl],
                                         start=(lk == 0), stop=(lk == NLK - 1))
                    # free ps_c the moment the last AV matmul retires:
                    # the copy is emitted BEFORE the final racc add so it
                    # does not queue behind it on DVE (the next head's
                    # first AV matmul WAR-waits on this copy)
                    if lk == NLK - 1 and h != H - 1:
                        ctx_raw = npool.tile([E, LQ], BF16, tag="ctx",
                                             name="ctx_raw")
                        nc.vector.tensor_copy(ctx_raw[:], ps_c[:])
                    # rowsum running accumulation on DVE (bf16 2x mode,
                    # ~700ns each, paced one per iteration by the exps)
                    if lk == 0:
                        racc = p_cur
                    else:
                        racc_new = rapool.tile([128, LQ], BF16, tag="racc",
                                               name="racc")
                        nc.vector.tensor_add(racc_new[:], racc[:], p_cur[:])
                        racc = racc_new

                # rowsum/norm happen next head; the final head's
                # normalization reads the psum tile directly in the epilogue
                if h == H - 1:
                    pending_tail[h] = (racc, ps_c)
                else:
                    pending_tail[h] = (racc, ctx_raw)

        # ---- epilogue: head 31's rowsum, then a short half-pipelined DVE
        # chain (normalize from psum, add the merged tree, bias, DMA out)
        emit_rowsum_prev()
        (hl, (ps_c31, recip31)), = pending_norm.items()
        pending_norm.clear()
        ctxn31 = npool.tile([E, LQ], BF16, tag="ctxn", name="ctxn")
        tfin = npool.tile([E, LQ], BF16, tag="merge", name="tfin")
        for half in range(2):
            sl = bass.ts(half, 512)
            nc.vector.tensor_mul(ctxn31[:, sl], ps_c31[:, sl],
                                 recip31[:, sl])
            nc.vector.tensor_add(tfin[:, sl], merged[:, sl], ctxn31[:, sl])
            nc.vector.tensor_scalar_add(out_acc[:, sl], tfin[:, sl],
                                        bo2_sb[:, 0:1])
            nc.sync.dma_start(outT[:, sl], out_acc[:, sl])


def build_program():
    nc = bacc.Bacc("TRN2", target_bir_lowering=False, debug=False,
                   num_devices=N_CORES)
    qT = nc.dram_tensor("qT", [E, LQ], BF16, kind="ExternalInput").ap()
    sT = nc.dram_tensor("sT", [D, LK], BF16, kind="ExternalInput").ap()
    wk = nc.dram_tensor("wk", [H, D, E], BF16, kind="ExternalInput").ap()
    u = nc.dram_tensor("u", [D, H * E], BF16, kind="ExternalInput").ap()
    ones = nc.dram_tensor("ones", [128, 128], BF16, kind="ExternalInput").ap()
    bo2 = nc.dram_tensor("bo2", [E, 1], F32, kind="ExternalInput").ap()
    outT = nc.dram_tensor("outT", [E, LQ], F32, kind="ExternalOutput").ap()

    with tile.TileContext(nc) as tc:
        _build_kernel(tc, qT, sT, wk, u, ones, bo2, outT)
    nc.compile()
    return nc


def make_in_maps(query, states, Wk, bk, Wv, bv, Wo, bo):
    """Shard the full inputs into per-core input maps (host-side prep)."""
    bb = ml_dtypes.bfloat16
    WoH = Wo.reshape(H, E, E).astype(np.float64)
    # fold Wo through the value projection and bv through the output bias
    # (softmax rows sum to 1), both exact in fp64
    U = np.einsum('hde,hef->hdf', Wv.astype(np.float64), WoH)
    u_packed = np.ascontiguousarray(
        np.transpose(U, (1, 0, 2)).reshape(D, H * E)).astype(bb)
    bo2 = bo.astype(np.float64) + np.einsum('he,hef->f',
                                            bv.astype(np.float64), WoH)
    bo2 = bo2.astype(np.float32).reshape(E, 1)
    wk_c = np.ascontiguousarray(Wk).astype(bb)
    ones_c = np.ones((128, 128), dtype=bb)

    in_maps = []
    for b in range(B):
        in_maps.append({
            "qT": np.ascontiguousarray(query[b].T).astype(bb),
            "sT": np.ascontiguousarray(states[b].T).astype(bb),
            "wk": wk_c,
            "u": u_packed,
            "ones": ones_c,
            "bo2": bo2,
        })
    return in_maps


_PROGRAM_CACHE = {}


def _get_program():
    if "nc" not in _PROGRAM_CACHE:
        _PROGRAM_CACHE["nc"] = build_program()
    return _PROGRAM_CACHE["nc"]


def kernel(query, states, Wk, bk, Wv, bv, Wo, bo, _trace=False, _tmpdir=None):
    args = [np.asarray(a, dtype=np.float32)
            for a in (query, states, Wk, bk, Wv, bv, Wo, bo)]
    nc = _get_program()
    in_maps = make_in_maps(*args)
    last_err = None
    for _attempt in range(2):  # one retry for transient device errors
        try:
            res = run_bass_kernel_spmd(nc, in_maps,
                                       core_ids=list(range(N_CORES)),
                                       trace=_trace, tmpdir=_tmpdir)
            break
        except Exception as e:  # noqa: BLE001
            last_err = e
    else:
        raise last_err
    out = np.stack([res.results[b]["outT"].T for b in range(B)])
    out = np.ascontiguousarray(out.astype(np.float32))
    if _trace:
        kernel.last_exec_time_ns = res.exec_time_ns
        kernel.last_results = res
    return out


if __name__ == "__main__":
    rng = np.random.default_rng(0)
    inputs = {
        "query": rng.standard_normal((B, LQ, E), dtype=np.float32),
        "states": rng.standard_normal((B, LK, D), dtype=np.float32),
        "Wk": rng.uniform(-0.04, 0.04, (H, D, E)).astype(np.float32),
        "bk": rng.uniform(-0.04, 0.04, (H, E)).astype(np.float32),
        "Wv": rng.uniform(-0.04, 0.04, (H, D, E)).astype(np.float32),
        "bv": rng.uniform(-0.04, 0.04, (H, E)).astype(np.float32),
        "Wo": rng.uniform(-0.015, 0.015, (H * E, E)).astype(np.float32),
        "bo": rng.uniform(-0.015, 0.015, (E,)).astype(np.float32),
    }
    out = kernel(**inputs)
    print(out.shape, out.dtype)
